# revision 6
# baseline (speedup 1.0000x reference)
"""DeepGATGNN Trainium2 kernel.

Strategy (edge-parallel, 8 cores): the dominant cost of this network is the
two per-edge [E,128]@[128,640] matmuls in each of the 5 GAT layers (~97% of
all FLOPs).  Those run on the 8 NeuronCores SPMD (5000 edges/core,
feature-major streams, bf16 inputs, fp32 PSUM accumulate).  The raw
pre-activations ship back to the host as fp8-e4m3 (halves the output DMA,
which the timeline shows is the kernel bottleneck); the host applies the
leaky-relu in f32, so quantization happens on the symmetric pre-activation
distribution where e4m3's normal range is best used.  The cheap glue
(leaky-relu, gathers of node features into edge endpoints, per-edge 10-way
head softmax, segment-sum aggregation, final graph pooling MLP) runs in
numpy between the 5 layer launches.
"""

import numpy as np
import ml_dtypes

bf16 = ml_dtypes.bfloat16
e4m3 = ml_dtypes.float8_e4m3

N, E, G = 10000, 40000, 128
NF, EF, H, NH, GD, L = 92, 50, 64, 10, 108, 5
EPS = 1e-5
NC = 8
E_SH = E // NC          # 5000 edges per core (exactly; no padding)
E_PAD = 5000
SLICES = (904, 2048, 2048)   # column slices; first is small so the packed
                             # head DMA lands quickly and the PE starts early
MCH = 5                 # 640 = 5 x 128 output-feature chunks
HEADW = 640 + SLICES[0]


def _lrelu(v):
    return np.where(v >= 0, v, np.float32(0.2) * v).astype(np.float32)


_NC_CACHE = {}


def _build_nc():
    import concourse.mybir as mybir
    from concourse import bacc, tile

    nc = bacc.Bacc(None, target_bir_lowering=False)
    dt = mybir.dt

    # head packs W with the first xi column slice so one DMA unblocks the PE
    head_d = nc.declare_dram_parameter("head", [128, HEADW], dt.bfloat16,
                                       isOutput=False)
    hr_d = nc.declare_dram_parameter("hr", [64, E_PAD], dt.bfloat16, isOutput=False)
    hc_d = nc.declare_dram_parameter("hc", [64, E_PAD], dt.bfloat16, isOutput=False)
    ea_d = nc.declare_dram_parameter("ea", [64, E_PAD], dt.bfloat16, isOutput=False)
    hi_d = nc.declare_dram_parameter("hi", [640, E_PAD], dt.float8e4, isOutput=True)
    hj_d = nc.declare_dram_parameter("hj", [640, E_PAD], dt.float8e4, isOutput=True)

    starts = []
    off = 0
    for w in SLICES:
        starts.append(off)
        off += w

    # greedy engine balance using the cost model's per-instruction times
    eng_t = {"A": 1283.0, "D": 0.0}   # ACT pays one act-table load

    def evac(dst, src, width):
        ca = eng_t["A"] + width * 0.833 + 185.0
        cd = eng_t["D"] + width * 1.042 + 125.0
        if ca <= cd:
            eng_t["A"] = ca
            nc.scalar.copy(dst, src)
        else:
            eng_t["D"] = cd
            nc.vector.tensor_copy(dst, src)

    with tile.TileContext(nc) as tc:
        with (
            tc.tile_pool(name="inp", bufs=1) as inp,
            tc.tile_pool(name="ps", bufs=2, space="PSUM") as ps,
            tc.tile_pool(name="ev", bufs=3) as ev,
        ):
            head_s = inp.tile([128, HEADW], dt.bfloat16, tag="head")
            nc.sync.dma_start(head_s[:], head_d[:])
            w_s = head_s[:, 0:640]
            # per-slice input tiles so matmuls start as soon as each column
            # slice has landed; xi slice 0 rides in with the head DMA
            x_sl = {("i", 0): head_s[:, 640:HEADW]}
            for side, top_d in (("i", hr_d), ("j", hc_d)):
                for t in range(3):
                    if (side, t) in x_sl:
                        continue
                    sl = slice(starts[t], starts[t] + SLICES[t])
                    x_t = inp.tile([128, SLICES[t]], dt.bfloat16,
                                   tag=f"x{side}{t}")
                    nc.sync.dma_start(x_t[0:64, :], top_d[:, sl])
                    nc.sync.dma_start(x_t[64:128, :], ea_d[:, sl])
                    x_sl[side, t] = x_t[:, :]

            for side, o_d in (("i", hi_d), ("j", hj_d)):
                for m in range(MCH):
                    o_strip = ev.tile([128, E_PAD], dt.float8e4, tag="o")
                    for t in range(3):
                        tw = SLICES[t]
                        acc = ps.tile([128, 2048], dt.float32, tag="acc")
                        for u0 in range(0, tw, 512):
                            u1 = min(u0 + 512, tw)
                            nc.tensor.matmul(
                                acc[:, u0:u1],
                                w_s[:, m * 128:(m + 1) * 128],
                                x_sl[side, t][:, u0:u1],
                                start=True,
                                stop=True,
                            )
                        # raw pre-activation evac with fp8 cast; host
                        # applies the leaky relu in f32
                        evac(o_strip[:, starts[t]:starts[t] + tw],
                             acc[:, 0:tw], tw)
                        if side == "j" and m == MCH - 1 and t >= 1:
                            # split the final strip's writeback so the
                            # launch tail is one small DMA, not a full strip
                            lo = 0 if t == 1 else starts[2]
                            hi = starts[t] + tw
                            nc.sync.dma_start(
                                o_d[m * 128:(m + 1) * 128, lo:hi],
                                o_strip[:, lo:hi])
                    if not (side == "j" and m == MCH - 1):
                        nc.sync.dma_start(
                            o_d[m * 128:(m + 1) * 128, :], o_strip[:])
    nc.compile()
    return nc


def _get_nc():
    if "nc" not in _NC_CACHE:
        _NC_CACHE["nc"] = _build_nc()
    return _NC_CACHE["nc"]


_EXEC_NS = 0
_EXEC_TIMES = []


def _get_runner():
    """Compile-once SPMD runner (same machinery run_bass_kernel_spmd uses
    under axon, but with the jitted executable cached across launches)."""
    if "runner" in _NC_CACHE:
        return _NC_CACHE["runner"]
    import jax
    import concourse.mybir as mybir
    from concourse import bass2jax
    from jax.sharding import Mesh, PartitionSpec
    from jax.experimental.shard_map import shard_map

    nc = _get_nc()
    bass2jax.install_neuronx_cc_hook()
    in_names, out_names, out_avals, zero_outs = [], [], [], []
    for alloc in nc.m.functions[0].allocations:
        if not isinstance(alloc, mybir.MemoryLocationSet):
            continue
        name = alloc.memorylocations[0].name
        if alloc.kind == "ExternalInput":
            in_names.append(name)
        elif alloc.kind == "ExternalOutput":
            out_names.append(name)
            shape = tuple(alloc.tensor_shape)
            dtype = mybir.dt.np(alloc.dtype)
            out_avals.append(jax.core.ShapedArray(shape, dtype))
            zero_outs.append(np.zeros((NC * shape[0], *shape[1:]), dtype))
    n_params = len(in_names)
    all_names = tuple(in_names + out_names)
    donate = tuple(range(n_params, n_params + len(out_names)))

    def _body(*args):
        outs = bass2jax._bass_exec_p.bind(
            *args,
            out_avals=tuple(out_avals),
            in_names=all_names,
            out_names=tuple(out_names),
            lowering_input_output_aliases=(),
            sim_require_finite=True,
            sim_require_nnan=True,
            nc=nc,
        )
        return tuple(outs)

    devices = jax.devices()[:NC]
    mesh = Mesh(np.asarray(devices), ("core",))
    specs = (PartitionSpec("core"),) * (n_params + len(out_names))
    sharded = jax.jit(
        shard_map(_body, mesh=mesh, in_specs=specs,
                  out_specs=(PartitionSpec("core"),) * len(out_names),
                  check_rep=False),
        donate_argnums=donate, keep_unused=True,
    )

    from jax.sharding import NamedSharding

    sharding = NamedSharding(mesh, PartitionSpec("core"))

    def run(in_maps):
        concat_in = []
        for name in in_names:
            if name == "ea" and "ea_dev" in _NC_CACHE:
                concat_in.append(_NC_CACHE["ea_dev"])
            else:
                concat_in.append(np.concatenate(
                    [np.asarray(m[name]) for m in in_maps], axis=0))
        zo = [np.zeros_like(z) for z in zero_outs]
        out_arrs = sharded(*concat_in, *zo)
        return [
            {
                name: np.asarray(out_arrs[i]).reshape(
                    NC, *out_avals[i].shape)[c]
                for i, name in enumerate(out_names)
            }
            for c in range(NC)
        ]

    def put_ea(ea_global):
        import jax
        _NC_CACHE["ea_dev"] = jax.device_put(ea_global, sharding)

    _NC_CACHE["put_ea"] = put_ea
    _NC_CACHE["runner"] = run
    return run


def _set_ea(ea):
    """ea: [E,64] f32 — upload the per-core ea^T shards once per kernel call."""
    _get_runner()
    ea_g = np.zeros((NC * 64, E_PAD), dtype=bf16)
    for c in range(NC):
        ea_g[c * 64:(c + 1) * 64, :E_SH] = \
            ea[c * E_SH:(c + 1) * E_SH].T.astype(bf16)
    _NC_CACHE["ea_host"] = ea_g
    _NC_CACHE["put_ea"](ea_g)


def _run_edge_mm(hrow, hcol, W):
    """hrow, hcol: [E,64] f32 gathered node feats; W: [128,640] f32
    -> hi, hj [E,640] f32 raw pre-activations."""
    import os

    from concourse.bass_utils import run_bass_kernel_spmd

    global _EXEC_NS
    nc = _get_nc()
    w_b = W.astype(bf16)
    ea_z = np.zeros((64, E_PAD), dtype=bf16)
    eah = _NC_CACHE.get("ea_host")
    in_maps = []
    for c in range(NC):
        sl = slice(c * E_SH, (c + 1) * E_SH)
        hr_t = np.ascontiguousarray(hrow[sl].T.astype(bf16))
        hc_t = np.ascontiguousarray(hcol[sl].T.astype(bf16))
        head = np.empty((128, HEADW), dtype=bf16)
        head[:, :640] = w_b
        head[0:64, 640:] = hr_t[:, :SLICES[0]]
        head[64:128, 640:] = eah[c * 64:(c + 1) * 64, :SLICES[0]]
        in_maps.append({"head": head, "hr": hr_t, "hc": hc_t, "ea": ea_z})
    try:
        res = _get_runner()(in_maps)
    except Exception:
        if eah is not None:
            for c in range(NC):
                in_maps[c]["ea"] = np.ascontiguousarray(
                    eah[c * 64:(c + 1) * 64])
        out = run_bass_kernel_spmd(nc, in_maps, list(range(NC)))
        res = out.results
    if os.environ.get("KERNEL_PROFILE"):
        if "sim_ns" not in _NC_CACHE:
            try:
                from concourse.timeline_sim import TimelineSim
                _NC_CACHE["sim_ns"] = float(TimelineSim(nc).simulate())
            except Exception:
                _NC_CACHE["sim_ns"] = 0.0
        _EXEC_NS += int(_NC_CACHE["sim_ns"])
        _EXEC_TIMES.append(int(_NC_CACHE["sim_ns"]))
    hi = np.empty((E, 640), np.float32)
    hj = np.empty((E, 640), np.float32)
    for c in range(NC):
        sl = slice(c * E_SH, (c + 1) * E_SH)
        hi[sl] = np.asarray(res[c]["hi"]).astype(np.float32)[:, :E_SH].T
        hj[sl] = np.asarray(res[c]["hj"]).astype(np.float32)[:, :E_SH].T
    return hi, hj


def _segsum(vals, idx, n):
    """vals [M,D] f32 summed into [n,D] by idx."""
    out = np.zeros((n, vals.shape[1]), np.float32)
    np.add.at(out, idx, vals)
    return out


def kernel(x, edge_index, edge_attr, batch_idx, global_features,
           node_W, node_b, edge_W, edge_b,
           conv_W, conv_att, conv_b, conv_gamma, conv_beta,
           ga_W1, ga_b1, ga_W2, ga_b2, out_W1, out_b1, out_W2, out_b2):
    f32 = np.float32
    x = np.asarray(x, f32)
    edge_index = np.asarray(edge_index)
    row = edge_index[0].astype(np.int64)
    col = edge_index[1].astype(np.int64)
    edge_attr = np.asarray(edge_attr, f32)
    batch_idx_np = np.asarray(batch_idx).astype(np.int64)
    gf = np.asarray(global_features, f32)
    conv_W = np.asarray(conv_W, f32)
    conv_att = np.asarray(conv_att, f32)
    conv_b = np.asarray(conv_b, f32)
    conv_gamma = np.asarray(conv_gamma, f32)
    conv_beta = np.asarray(conv_beta, f32)

    h = _lrelu(x @ np.asarray(node_W, f32) + np.asarray(node_b, f32))
    ea = _lrelu(edge_attr @ np.asarray(edge_W, f32) + np.asarray(edge_b, f32))
    initial = h.copy()
    inv_std = f32(1.0 / np.sqrt(1.0 + EPS))

    _set_ea(ea)
    for i in range(L):
        hi, hj = _run_edge_mm(h[row], h[col], conv_W[i])  # raw pre-activations
        hi = np.maximum(hi, f32(0.2) * hi).reshape(E, NH, H)
        hj = np.maximum(hj, f32(0.2) * hj).reshape(E, NH, H)
        att = conv_att[i]                            # [NH, 2H]
        alpha = (hi * att[None, :, :H]).sum(-1) + (hj * att[None, :, H:]).sum(-1)
        alpha = _lrelu(alpha)
        alpha = alpha * inv_std * conv_gamma[i] + conv_beta[i]
        alpha = alpha - alpha.max(axis=1, keepdims=True)
        ex = np.exp(alpha)
        alpha = ex / ex.sum(axis=1, keepdims=True)
        msum = (hj * alpha[..., None]).mean(axis=1)  # [E,64] head-mean of messages
        agg = _segsum(msum, row, N)
        h_new = agg + conv_b[i]
        h = h + h_new if i > 0 else h_new
    h = h + initial

    # global attention pooling
    g = gf[batch_idx_np]
    s = _lrelu(np.concatenate([h, g], axis=1) @ np.asarray(ga_W1, f32)
               + np.asarray(ga_b1, f32))
    score = (s @ np.asarray(ga_W2, f32) + np.asarray(ga_b2, f32))[:, 0]
    smax = np.full(G, -np.inf, f32)
    np.maximum.at(smax, batch_idx_np, score)
    ex = np.exp(score - smax[batch_idx_np])
    denom = np.zeros(G, f32)
    np.add.at(denom, batch_idx_np, ex)
    w = (ex / denom[batch_idx_np])[:, None]
    pooled = _segsum(h * w, batch_idx_np, G)
    out = (np.maximum(pooled @ np.asarray(out_W1, f32) + np.asarray(out_b1, f32), 0)
           @ np.asarray(out_W2, f32) + np.asarray(out_b2, f32))
    return out[:, 0].astype(np.float32)


# revision 9
# speedup vs baseline: 1.3168x; 1.3168x over previous
"""DeepGATGNN Trainium2 kernel.

Strategy (edge-parallel, 8 cores): the dominant cost of this network is the
two per-edge [E,128]@[128,640] matmuls in each of the 5 GAT layers (~97% of
all FLOPs).  Those run on the 8 NeuronCores SPMD (5000 edges/core,
feature-major streams, bf16 inputs, fp32 PSUM accumulate).  The raw
pre-activations ship back to the host as fp8-e4m3 (halves the output DMA,
which the timeline shows is the kernel bottleneck); the host applies the
leaky-relu in f32, so quantization happens on the symmetric pre-activation
distribution where e4m3's normal range is best used.  The cheap glue
(leaky-relu, gathers of node features into edge endpoints, per-edge 10-way
head softmax, segment-sum aggregation, final graph pooling MLP) runs in
numpy between the 5 layer launches.
"""

import numpy as np
import ml_dtypes

bf16 = ml_dtypes.bfloat16
e4m3 = ml_dtypes.float8_e4m3

N, E, G = 10000, 40000, 128
NF, EF, H, NH, GD, L = 92, 50, 64, 10, 108, 5
EPS = 1e-5
NC = 8
E_SH = E // NC          # 5000 edges per core (exactly; no padding)
E_PAD = 5000
SLICES = (904, 1024, 1024, 1024, 1024)   # column slices; first is small so
                             # the packed head DMA lands quickly
MCH = 5                 # 640 = 5 x 128 output-feature chunks
HEADW = 640 + SLICES[0]


def _lrelu(v):
    return np.where(v >= 0, v, np.float32(0.2) * v).astype(np.float32)


_NC_CACHE = {}


def _build_nc():
    import concourse.mybir as mybir
    from concourse import bacc, tile

    nc = bacc.Bacc(None, target_bir_lowering=False)
    dt = mybir.dt

    # head packs W with the first xi column slice so one DMA unblocks the PE
    head_d = nc.declare_dram_parameter("head", [128, HEADW], dt.bfloat16,
                                       isOutput=False)
    hr_d = nc.declare_dram_parameter("hr", [64, E_PAD], dt.bfloat16, isOutput=False)
    hc_d = nc.declare_dram_parameter("hc", [64, E_PAD], dt.bfloat16, isOutput=False)
    ea_d = nc.declare_dram_parameter("ea", [64, E_PAD], dt.bfloat16, isOutput=False)
    hi_d = nc.declare_dram_parameter("hi", [640, E_PAD], dt.float8e4, isOutput=True)
    hj_d = nc.declare_dram_parameter("hj", [640, E_PAD], dt.float8e4, isOutput=True)

    starts = []
    off = 0
    for w in SLICES:
        starts.append(off)
        off += w

    # greedy engine balance using the cost model's per-instruction times
    eng_t = {"A": 1283.0, "D": 0.0}   # ACT pays one act-table load

    def evac(dst, src, width):
        ca = eng_t["A"] + width * 0.833 + 185.0
        cd = eng_t["D"] + width * 1.042 + 125.0
        if ca <= cd:
            eng_t["A"] = ca
            nc.scalar.copy(dst, src)
        else:
            eng_t["D"] = cd
            nc.vector.tensor_copy(dst, src)

    NSL = len(SLICES)
    with tile.TileContext(nc) as tc:
        with (
            tc.tile_pool(name="inp", bufs=1) as inp,
            tc.tile_pool(name="ps", bufs=4, space="PSUM") as ps,
            tc.tile_pool(name="ev", bufs=7) as ev,
        ):
            head_s = inp.tile([128, HEADW], dt.bfloat16, tag="head")
            nc.sync.dma_start(head_s[:], head_d[:])
            w_s = head_s[:, 0:640]
            # per-slice input tiles so matmuls start as soon as each column
            # slice has landed; xi slice 0 rides in with the head DMA
            x_sl = {("i", 0): head_s[:, 640:HEADW]}
            for side, top_d in (("i", hr_d), ("j", hc_d)):
                for t in range(NSL):
                    if (side, t) in x_sl:
                        continue
                    sl = slice(starts[t], starts[t] + SLICES[t])
                    x_t = inp.tile([128, SLICES[t]], dt.bfloat16,
                                   tag=f"x{side}{t}")
                    nc.sync.dma_start(x_t[0:64, :], top_d[:, sl])
                    nc.sync.dma_start(x_t[64:128, :], ea_d[:, sl])
                    x_sl[side, t] = x_t[:, :]

            strips = {}

            def tilework(side, o_d, m, t):
                tw = SLICES[t]
                acc = ps.tile([128, 1024], dt.float32, tag="acc")
                for u0 in range(0, tw, 512):
                    u1 = min(u0 + 512, tw)
                    nc.tensor.matmul(
                        acc[:, u0:u1],
                        w_s[:, m * 128:(m + 1) * 128],
                        x_sl[side, t][:, u0:u1],
                        start=True,
                        stop=True,
                    )
                # raw pre-activation evac with fp8 cast; host applies the
                # leaky relu in f32
                evac(strips[side, m][:, starts[t]:starts[t] + tw],
                     acc[:, 0:tw], tw)
                last = side == "j" and m == MCH - 1
                if t == NSL - 1 and not last:
                    nc.sync.dma_start(
                        o_d[m * 128:(m + 1) * 128, :], strips[side, m][:])
                elif last and t >= NSL - 2:
                    # split the final strip's writeback so the launch tail
                    # is one small DMA, not a full strip
                    lo = 0 if t == NSL - 2 else starts[t]
                    hi = starts[t] + tw
                    nc.sync.dma_start(
                        o_d[m * 128:(m + 1) * 128, lo:hi],
                        strips[side, m][:, lo:hi])

            # side i: front-load all chunks of slice 0 (it arrives with the
            # head DMA) so the PE and the evac engines start immediately
            for m in range(MCH):
                strips["i", m] = ev.tile([128, E_PAD], dt.float8e4, tag="o", name=f"oi{m}")
                tilework("i", hi_d, m, 0)
            for m in range(MCH):
                for t in range(1, NSL):
                    tilework("i", hi_d, m, t)
            for m in range(MCH):
                strips["j", m] = ev.tile([128, E_PAD], dt.float8e4, tag="o", name=f"oj{m}")
                for t in range(NSL):
                    tilework("j", hj_d, m, t)
    nc.compile()
    return nc


def _get_nc():
    if "nc" not in _NC_CACHE:
        _NC_CACHE["nc"] = _build_nc()
    return _NC_CACHE["nc"]


_EXEC_NS = 0
_EXEC_TIMES = []


def _get_runner():
    """Compile-once SPMD runner (same machinery run_bass_kernel_spmd uses
    under axon, but with the jitted executable cached across launches)."""
    if "runner" in _NC_CACHE:
        return _NC_CACHE["runner"]
    import jax
    import concourse.mybir as mybir
    from concourse import bass2jax
    from jax.sharding import Mesh, PartitionSpec
    from jax.experimental.shard_map import shard_map

    nc = _get_nc()
    bass2jax.install_neuronx_cc_hook()
    in_names, out_names, out_avals, zero_outs = [], [], [], []
    for alloc in nc.m.functions[0].allocations:
        if not isinstance(alloc, mybir.MemoryLocationSet):
            continue
        name = alloc.memorylocations[0].name
        if alloc.kind == "ExternalInput":
            in_names.append(name)
        elif alloc.kind == "ExternalOutput":
            out_names.append(name)
            shape = tuple(alloc.tensor_shape)
            dtype = mybir.dt.np(alloc.dtype)
            out_avals.append(jax.core.ShapedArray(shape, dtype))
            zero_outs.append(np.zeros((NC * shape[0], *shape[1:]), dtype))
    n_params = len(in_names)
    all_names = tuple(in_names + out_names)
    donate = tuple(range(n_params, n_params + len(out_names)))

    def _body(*args):
        outs = bass2jax._bass_exec_p.bind(
            *args,
            out_avals=tuple(out_avals),
            in_names=all_names,
            out_names=tuple(out_names),
            lowering_input_output_aliases=(),
            sim_require_finite=True,
            sim_require_nnan=True,
            nc=nc,
        )
        return tuple(outs)

    devices = jax.devices()[:NC]
    mesh = Mesh(np.asarray(devices), ("core",))
    specs = (PartitionSpec("core"),) * (n_params + len(out_names))
    sharded = jax.jit(
        shard_map(_body, mesh=mesh, in_specs=specs,
                  out_specs=(PartitionSpec("core"),) * len(out_names),
                  check_rep=False),
        donate_argnums=donate, keep_unused=True,
    )

    from jax.sharding import NamedSharding

    sharding = NamedSharding(mesh, PartitionSpec("core"))

    def run(in_maps):
        concat_in = []
        for name in in_names:
            if name == "ea" and "ea_dev" in _NC_CACHE:
                concat_in.append(_NC_CACHE["ea_dev"])
            else:
                concat_in.append(np.concatenate(
                    [np.asarray(m[name]) for m in in_maps], axis=0))
        zo = [np.zeros_like(z) for z in zero_outs]
        out_arrs = sharded(*concat_in, *zo)
        return [
            {
                name: np.asarray(out_arrs[i]).reshape(
                    NC, *out_avals[i].shape)[c]
                for i, name in enumerate(out_names)
            }
            for c in range(NC)
        ]

    def put_ea(ea_global):
        import jax
        _NC_CACHE["ea_dev"] = jax.device_put(ea_global, sharding)

    _NC_CACHE["put_ea"] = put_ea
    _NC_CACHE["runner"] = run
    return run


def _set_ea(ea):
    """ea: [E,64] f32 — upload the per-core ea^T shards once per kernel call."""
    _get_runner()
    ea_g = np.zeros((NC * 64, E_PAD), dtype=bf16)
    for c in range(NC):
        ea_g[c * 64:(c + 1) * 64, :E_SH] = \
            ea[c * E_SH:(c + 1) * E_SH].T.astype(bf16)
    _NC_CACHE["ea_host"] = ea_g
    _NC_CACHE["put_ea"](ea_g)


def _run_edge_mm(hrow, hcol, W):
    """hrow, hcol: [E,64] f32 gathered node feats; W: [128,640] f32
    -> hi, hj [E,640] f32 raw pre-activations."""
    import os

    from concourse.bass_utils import run_bass_kernel_spmd

    global _EXEC_NS
    nc = _get_nc()
    w_b = W.astype(bf16)
    ea_z = np.zeros((64, E_PAD), dtype=bf16)
    eah = _NC_CACHE.get("ea_host")
    in_maps = []
    for c in range(NC):
        sl = slice(c * E_SH, (c + 1) * E_SH)
        hr_t = np.ascontiguousarray(hrow[sl].T.astype(bf16))
        hc_t = np.ascontiguousarray(hcol[sl].T.astype(bf16))
        head = np.empty((128, HEADW), dtype=bf16)
        head[:, :640] = w_b
        head[0:64, 640:] = hr_t[:, :SLICES[0]]
        head[64:128, 640:] = eah[c * 64:(c + 1) * 64, :SLICES[0]]
        in_maps.append({"head": head, "hr": hr_t, "hc": hc_t, "ea": ea_z})
    try:
        res = _get_runner()(in_maps)
    except Exception:
        if eah is not None:
            for c in range(NC):
                in_maps[c]["ea"] = np.ascontiguousarray(
                    eah[c * 64:(c + 1) * 64])
        out = run_bass_kernel_spmd(nc, in_maps, list(range(NC)))
        res = out.results
    if os.environ.get("KERNEL_PROFILE"):
        if "sim_ns" not in _NC_CACHE:
            try:
                from concourse.timeline_sim import TimelineSim
                _NC_CACHE["sim_ns"] = float(TimelineSim(nc).simulate())
            except Exception:
                _NC_CACHE["sim_ns"] = 0.0
        _EXEC_NS += int(_NC_CACHE["sim_ns"])
        _EXEC_TIMES.append(int(_NC_CACHE["sim_ns"]))
    hi = np.empty((E, 640), np.float32)
    hj = np.empty((E, 640), np.float32)
    for c in range(NC):
        sl = slice(c * E_SH, (c + 1) * E_SH)
        hi[sl] = np.asarray(res[c]["hi"]).astype(np.float32)[:, :E_SH].T
        hj[sl] = np.asarray(res[c]["hj"]).astype(np.float32)[:, :E_SH].T
    return hi, hj


def _segsum(vals, idx, n):
    """vals [M,D] f32 summed into [n,D] by idx."""
    out = np.zeros((n, vals.shape[1]), np.float32)
    np.add.at(out, idx, vals)
    return out


def kernel(x, edge_index, edge_attr, batch_idx, global_features,
           node_W, node_b, edge_W, edge_b,
           conv_W, conv_att, conv_b, conv_gamma, conv_beta,
           ga_W1, ga_b1, ga_W2, ga_b2, out_W1, out_b1, out_W2, out_b2):
    f32 = np.float32
    x = np.asarray(x, f32)
    edge_index = np.asarray(edge_index)
    row = edge_index[0].astype(np.int64)
    col = edge_index[1].astype(np.int64)
    edge_attr = np.asarray(edge_attr, f32)
    batch_idx_np = np.asarray(batch_idx).astype(np.int64)
    gf = np.asarray(global_features, f32)
    conv_W = np.asarray(conv_W, f32)
    conv_att = np.asarray(conv_att, f32)
    conv_b = np.asarray(conv_b, f32)
    conv_gamma = np.asarray(conv_gamma, f32)
    conv_beta = np.asarray(conv_beta, f32)

    h = _lrelu(x @ np.asarray(node_W, f32) + np.asarray(node_b, f32))
    ea = _lrelu(edge_attr @ np.asarray(edge_W, f32) + np.asarray(edge_b, f32))
    initial = h.copy()
    inv_std = f32(1.0 / np.sqrt(1.0 + EPS))

    _set_ea(ea)
    for i in range(L):
        hi, hj = _run_edge_mm(h[row], h[col], conv_W[i])  # raw pre-activations
        hi = np.maximum(hi, f32(0.2) * hi).reshape(E, NH, H)
        hj = np.maximum(hj, f32(0.2) * hj).reshape(E, NH, H)
        att = conv_att[i]                            # [NH, 2H]
        alpha = (hi * att[None, :, :H]).sum(-1) + (hj * att[None, :, H:]).sum(-1)
        alpha = _lrelu(alpha)
        alpha = alpha * inv_std * conv_gamma[i] + conv_beta[i]
        alpha = alpha - alpha.max(axis=1, keepdims=True)
        ex = np.exp(alpha)
        alpha = ex / ex.sum(axis=1, keepdims=True)
        msum = (hj * alpha[..., None]).mean(axis=1)  # [E,64] head-mean of messages
        agg = _segsum(msum, row, N)
        h_new = agg + conv_b[i]
        h = h + h_new if i > 0 else h_new
    h = h + initial

    # global attention pooling
    g = gf[batch_idx_np]
    s = _lrelu(np.concatenate([h, g], axis=1) @ np.asarray(ga_W1, f32)
               + np.asarray(ga_b1, f32))
    score = (s @ np.asarray(ga_W2, f32) + np.asarray(ga_b2, f32))[:, 0]
    smax = np.full(G, -np.inf, f32)
    np.maximum.at(smax, batch_idx_np, score)
    ex = np.exp(score - smax[batch_idx_np])
    denom = np.zeros(G, f32)
    np.add.at(denom, batch_idx_np, ex)
    w = (ex / denom[batch_idx_np])[:, None]
    pooled = _segsum(h * w, batch_idx_np, G)
    out = (np.maximum(pooled @ np.asarray(out_W1, f32) + np.asarray(out_b1, f32), 0)
           @ np.asarray(out_W2, f32) + np.asarray(out_b2, f32))
    return out[:, 0].astype(np.float32)


# revision 12
# speedup vs baseline: 1.3750x; 1.0442x over previous
"""DeepGATGNN Trainium2 kernel.

Strategy (edge-parallel, 8 cores): the dominant cost of this network is the
two per-edge [E,128]@[128,640] matmuls in each of the 5 GAT layers (~97% of
all FLOPs).  Those run on the 8 NeuronCores SPMD (5000 edges/core,
feature-major streams, bf16 inputs, fp32 PSUM accumulate).  The raw
pre-activations ship back to the host as fp8-e4m3 (halves the output DMA,
which the timeline shows is the kernel bottleneck); the host applies the
leaky-relu in f32, so quantization happens on the symmetric pre-activation
distribution where e4m3's normal range is best used.  The cheap glue
(leaky-relu, gathers of node features into edge endpoints, per-edge 10-way
head softmax, segment-sum aggregation, final graph pooling MLP) runs in
numpy between the 5 layer launches.
"""

import numpy as np
import ml_dtypes

bf16 = ml_dtypes.bfloat16
e4m3 = ml_dtypes.float8_e4m3

N, E, G = 10000, 40000, 128
NF, EF, H, NH, GD, L = 92, 50, 64, 10, 108, 5
EPS = 1e-5
NC = 8
E_SH = E // NC          # 5000 edges per core (exactly; no padding)
E_PAD = 5000
SLICES = (904, 1024, 1024, 1024, 1024)   # column slices; first is small so
                             # the packed head DMA lands quickly
MCH = 5                 # 640 = 5 x 128 output-feature chunks
HEADW = 640 + SLICES[0]


def _lrelu(v):
    return np.where(v >= 0, v, np.float32(0.2) * v).astype(np.float32)


_NC_CACHE = {}


def _build_nc():
    import concourse.mybir as mybir
    from concourse import bacc, tile

    nc = bacc.Bacc(None, target_bir_lowering=False)
    dt = mybir.dt

    # head packs W with the first xi column slice so one DMA unblocks the PE
    head_d = nc.declare_dram_parameter("head", [128, HEADW], dt.bfloat16,
                                       isOutput=False)
    hr_d = nc.declare_dram_parameter("hr", [64, E_PAD], dt.bfloat16, isOutput=False)
    hc_d = nc.declare_dram_parameter("hc", [64, E_PAD], dt.bfloat16, isOutput=False)
    ea_d = nc.declare_dram_parameter("ea", [64, E_PAD], dt.bfloat16, isOutput=False)
    hi_d = nc.declare_dram_parameter("hi", [640, E_PAD], dt.float8e4, isOutput=True)
    hj_d = nc.declare_dram_parameter("hj", [640, E_PAD], dt.float8e4, isOutput=True)

    starts = []
    off = 0
    for w in SLICES:
        starts.append(off)
        off += w

    # greedy engine balance using the cost model's per-instruction times
    # (the one-time ACT table load runs during the input-DMA head, so it
    # does not count against ACT's stream)
    eng_t = {"A": 0.0, "D": 0.0}

    def evac(dst, src, width):
        ca = eng_t["A"] + width * 0.833 + 185.0
        cd = eng_t["D"] + width * 1.042 + 125.0
        if ca <= cd:
            eng_t["A"] = ca
            nc.scalar.copy(dst, src)
        else:
            eng_t["D"] = cd
            nc.vector.tensor_copy(dst, src)

    NSL = len(SLICES)
    with tile.TileContext(nc) as tc:
        with (
            tc.tile_pool(name="inp", bufs=1) as inp,
            tc.tile_pool(name="ps", bufs=4, space="PSUM") as ps,
            tc.tile_pool(name="ev", bufs=7) as ev,
        ):
            head_s = inp.tile([128, HEADW], dt.bfloat16, tag="head")
            # warm-up: the PE p-state ramp needs ~3us of continuous busy to
            # reach 2.4GHz; zero matmuls from t~0.6us finish the ramp right
            # as the first real operands land, so real matmuls all run at
            # full clock
            dummy_s = inp.tile([128, 512], dt.bfloat16, tag="dummy")
            nc.gpsimd.memset(dummy_s[:], 0)
            dacc = ps.tile([128, 1024], dt.float32, tag="acc")
            for _ in range(9):
                nc.tensor.matmul(dacc[:, 0:512], dummy_s[:, 0:128],
                                 dummy_s[:], start=True, stop=True)
            nc.sync.dma_start(head_s[:], head_d[:])
            w_s = head_s[:, 0:640]
            # per-slice input tiles so matmuls start as soon as each column
            # slice has landed; xi slice 0 rides in with the head DMA
            x_sl = {("i", 0): head_s[:, 640:HEADW]}
            for side, top_d in (("i", hr_d), ("j", hc_d)):
                for t in range(NSL):
                    if (side, t) in x_sl:
                        continue
                    sl = slice(starts[t], starts[t] + SLICES[t])
                    x_t = inp.tile([128, SLICES[t]], dt.bfloat16,
                                   tag=f"x{side}{t}")
                    nc.sync.dma_start(x_t[0:64, :], top_d[:, sl])
                    nc.sync.dma_start(x_t[64:128, :], ea_d[:, sl])
                    x_sl[side, t] = x_t[:, :]

            strips = {}

            def tilework(side, o_d, m, t):
                tw = SLICES[t]
                acc = ps.tile([128, 1024], dt.float32, tag="acc")
                for u0 in range(0, tw, 512):
                    u1 = min(u0 + 512, tw)
                    nc.tensor.matmul(
                        acc[:, u0:u1],
                        w_s[:, m * 128:(m + 1) * 128],
                        x_sl[side, t][:, u0:u1],
                        start=True,
                        stop=True,
                    )
                # raw pre-activation evac with fp8 cast; host applies the
                # leaky relu in f32
                evac(strips[side, m][:, starts[t]:starts[t] + tw],
                     acc[:, 0:tw], tw)
                if side == "j" and m >= MCH - 2:
                    # the last two strips finish when the evac stream is
                    # nearly done: write back per-slice so only one small
                    # DMA trails the final evac instead of full strips
                    nc.sync.dma_start(
                        o_d[m * 128:(m + 1) * 128, starts[t]:starts[t] + tw],
                        strips[side, m][:, starts[t]:starts[t] + tw])
                elif t == NSL - 1:
                    nc.sync.dma_start(
                        o_d[m * 128:(m + 1) * 128, :], strips[side, m][:])

            # side i: front-load all chunks of slice 0 (it arrives with the
            # head DMA) so the PE and the evac engines start immediately
            for m in range(MCH):
                strips["i", m] = ev.tile([128, E_PAD], dt.float8e4, tag="o", name=f"oi{m}")
                tilework("i", hi_d, m, 0)
            for m in range(MCH):
                for t in range(1, NSL):
                    tilework("i", hi_d, m, t)
            for m in range(MCH):
                strips["j", m] = ev.tile([128, E_PAD], dt.float8e4, tag="o", name=f"oj{m}")
                for t in range(NSL):
                    tilework("j", hj_d, m, t)
    nc.compile()
    return nc


def _get_nc():
    if "nc" not in _NC_CACHE:
        _NC_CACHE["nc"] = _build_nc()
    return _NC_CACHE["nc"]


_EXEC_NS = 0
_EXEC_TIMES = []


def _get_runner():
    """Compile-once SPMD runner (same machinery run_bass_kernel_spmd uses
    under axon, but with the jitted executable cached across launches)."""
    if "runner" in _NC_CACHE:
        return _NC_CACHE["runner"]
    import jax
    import concourse.mybir as mybir
    from concourse import bass2jax
    from jax.sharding import Mesh, PartitionSpec
    from jax.experimental.shard_map import shard_map

    nc = _get_nc()
    bass2jax.install_neuronx_cc_hook()
    in_names, out_names, out_avals, zero_outs = [], [], [], []
    for alloc in nc.m.functions[0].allocations:
        if not isinstance(alloc, mybir.MemoryLocationSet):
            continue
        name = alloc.memorylocations[0].name
        if alloc.kind == "ExternalInput":
            in_names.append(name)
        elif alloc.kind == "ExternalOutput":
            out_names.append(name)
            shape = tuple(alloc.tensor_shape)
            dtype = mybir.dt.np(alloc.dtype)
            out_avals.append(jax.core.ShapedArray(shape, dtype))
            zero_outs.append(np.zeros((NC * shape[0], *shape[1:]), dtype))
    n_params = len(in_names)
    all_names = tuple(in_names + out_names)
    donate = tuple(range(n_params, n_params + len(out_names)))

    def _body(*args):
        outs = bass2jax._bass_exec_p.bind(
            *args,
            out_avals=tuple(out_avals),
            in_names=all_names,
            out_names=tuple(out_names),
            lowering_input_output_aliases=(),
            sim_require_finite=True,
            sim_require_nnan=True,
            nc=nc,
        )
        return tuple(outs)

    devices = jax.devices()[:NC]
    mesh = Mesh(np.asarray(devices), ("core",))
    specs = (PartitionSpec("core"),) * (n_params + len(out_names))
    sharded = jax.jit(
        shard_map(_body, mesh=mesh, in_specs=specs,
                  out_specs=(PartitionSpec("core"),) * len(out_names),
                  check_rep=False),
        donate_argnums=donate, keep_unused=True,
    )

    from jax.sharding import NamedSharding

    sharding = NamedSharding(mesh, PartitionSpec("core"))

    def run(in_maps):
        concat_in = []
        for name in in_names:
            if name == "ea" and "ea_dev" in _NC_CACHE:
                concat_in.append(_NC_CACHE["ea_dev"])
            else:
                concat_in.append(np.concatenate(
                    [np.asarray(m[name]) for m in in_maps], axis=0))
        zo = [np.zeros_like(z) for z in zero_outs]
        out_arrs = sharded(*concat_in, *zo)
        return [
            {
                name: np.asarray(out_arrs[i]).reshape(
                    NC, *out_avals[i].shape)[c]
                for i, name in enumerate(out_names)
            }
            for c in range(NC)
        ]

    def put_ea(ea_global):
        import jax
        _NC_CACHE["ea_dev"] = jax.device_put(ea_global, sharding)

    _NC_CACHE["put_ea"] = put_ea
    _NC_CACHE["runner"] = run
    return run


def _set_ea(ea):
    """ea: [E,64] f32 — upload the per-core ea^T shards once per kernel call."""
    _get_runner()
    ea_g = np.zeros((NC * 64, E_PAD), dtype=bf16)
    for c in range(NC):
        ea_g[c * 64:(c + 1) * 64, :E_SH] = \
            ea[c * E_SH:(c + 1) * E_SH].T.astype(bf16)
    _NC_CACHE["ea_host"] = ea_g
    _NC_CACHE["put_ea"](ea_g)


def _run_edge_mm(hrow, hcol, W):
    """hrow, hcol: [E,64] f32 gathered node feats; W: [128,640] f32
    -> hi, hj [E,640] f32 raw pre-activations."""
    import os

    from concourse.bass_utils import run_bass_kernel_spmd

    global _EXEC_NS
    nc = _get_nc()
    w_b = W.astype(bf16)
    ea_z = np.zeros((64, E_PAD), dtype=bf16)
    eah = _NC_CACHE.get("ea_host")
    in_maps = []
    for c in range(NC):
        sl = slice(c * E_SH, (c + 1) * E_SH)
        hr_t = np.ascontiguousarray(hrow[sl].T.astype(bf16))
        hc_t = np.ascontiguousarray(hcol[sl].T.astype(bf16))
        head = np.empty((128, HEADW), dtype=bf16)
        head[:, :640] = w_b
        head[0:64, 640:] = hr_t[:, :SLICES[0]]
        head[64:128, 640:] = eah[c * 64:(c + 1) * 64, :SLICES[0]]
        in_maps.append({"head": head, "hr": hr_t, "hc": hc_t, "ea": ea_z})
    try:
        res = _get_runner()(in_maps)
    except Exception:
        if eah is not None:
            for c in range(NC):
                in_maps[c]["ea"] = np.ascontiguousarray(
                    eah[c * 64:(c + 1) * 64])
        out = run_bass_kernel_spmd(nc, in_maps, list(range(NC)))
        res = out.results
    if os.environ.get("KERNEL_PROFILE"):
        if "sim_ns" not in _NC_CACHE:
            try:
                from concourse.timeline_sim import TimelineSim
                _NC_CACHE["sim_ns"] = float(TimelineSim(nc).simulate())
            except Exception:
                _NC_CACHE["sim_ns"] = 0.0
        _EXEC_NS += int(_NC_CACHE["sim_ns"])
        _EXEC_TIMES.append(int(_NC_CACHE["sim_ns"]))
    hi = np.empty((E, 640), np.float32)
    hj = np.empty((E, 640), np.float32)
    for c in range(NC):
        sl = slice(c * E_SH, (c + 1) * E_SH)
        hi[sl] = np.asarray(res[c]["hi"]).astype(np.float32)[:, :E_SH].T
        hj[sl] = np.asarray(res[c]["hj"]).astype(np.float32)[:, :E_SH].T
    return hi, hj


def _segsum(vals, idx, n):
    """vals [M,D] f32 summed into [n,D] by idx."""
    out = np.zeros((n, vals.shape[1]), np.float32)
    np.add.at(out, idx, vals)
    return out


def kernel(x, edge_index, edge_attr, batch_idx, global_features,
           node_W, node_b, edge_W, edge_b,
           conv_W, conv_att, conv_b, conv_gamma, conv_beta,
           ga_W1, ga_b1, ga_W2, ga_b2, out_W1, out_b1, out_W2, out_b2):
    f32 = np.float32
    x = np.asarray(x, f32)
    edge_index = np.asarray(edge_index)
    row = edge_index[0].astype(np.int64)
    col = edge_index[1].astype(np.int64)
    edge_attr = np.asarray(edge_attr, f32)
    batch_idx_np = np.asarray(batch_idx).astype(np.int64)
    gf = np.asarray(global_features, f32)
    conv_W = np.asarray(conv_W, f32)
    conv_att = np.asarray(conv_att, f32)
    conv_b = np.asarray(conv_b, f32)
    conv_gamma = np.asarray(conv_gamma, f32)
    conv_beta = np.asarray(conv_beta, f32)

    h = _lrelu(x @ np.asarray(node_W, f32) + np.asarray(node_b, f32))
    ea = _lrelu(edge_attr @ np.asarray(edge_W, f32) + np.asarray(edge_b, f32))
    initial = h.copy()
    inv_std = f32(1.0 / np.sqrt(1.0 + EPS))

    _set_ea(ea)
    for i in range(L):
        hi, hj = _run_edge_mm(h[row], h[col], conv_W[i])  # raw pre-activations
        hi = np.maximum(hi, f32(0.2) * hi).reshape(E, NH, H)
        hj = np.maximum(hj, f32(0.2) * hj).reshape(E, NH, H)
        att = conv_att[i]                            # [NH, 2H]
        alpha = (hi * att[None, :, :H]).sum(-1) + (hj * att[None, :, H:]).sum(-1)
        alpha = _lrelu(alpha)
        alpha = alpha * inv_std * conv_gamma[i] + conv_beta[i]
        alpha = alpha - alpha.max(axis=1, keepdims=True)
        ex = np.exp(alpha)
        alpha = ex / ex.sum(axis=1, keepdims=True)
        msum = (hj * alpha[..., None]).mean(axis=1)  # [E,64] head-mean of messages
        agg = _segsum(msum, row, N)
        h_new = agg + conv_b[i]
        h = h + h_new if i > 0 else h_new
    h = h + initial

    # global attention pooling
    g = gf[batch_idx_np]
    s = _lrelu(np.concatenate([h, g], axis=1) @ np.asarray(ga_W1, f32)
               + np.asarray(ga_b1, f32))
    score = (s @ np.asarray(ga_W2, f32) + np.asarray(ga_b2, f32))[:, 0]
    smax = np.full(G, -np.inf, f32)
    np.maximum.at(smax, batch_idx_np, score)
    ex = np.exp(score - smax[batch_idx_np])
    denom = np.zeros(G, f32)
    np.add.at(denom, batch_idx_np, ex)
    w = (ex / denom[batch_idx_np])[:, None]
    pooled = _segsum(h * w, batch_idx_np, G)
    out = (np.maximum(pooled @ np.asarray(out_W1, f32) + np.asarray(out_b1, f32), 0)
           @ np.asarray(out_W2, f32) + np.asarray(out_b2, f32))
    return out[:, 0].astype(np.float32)
